# revision 1
# baseline (speedup 1.0000x reference)
"""DA-RNN style encoder (LSTM + input attention) on 8 Trainium2 cores.

Problem: nn_Encoder_63024350101963
  B=2048, T-1=31 steps, D=128 input feats, H=128 hidden.

Key algebraic fact exploited: in the reference,
    score = (h @ w_h + c @ w_c + b)[:, None] + x_score
the recurrent term is constant along the softmax axis, and softmax is
shift-invariant, so
    attn = softmax(x_score)      (time-constant, recurrence-independent)
Therefore weighted = attn[:,None,:] * x  is a pure elementwise op and only
the LSTM cell recurrence is serial.

Device layout: feature-on-partitions, batch-on-free ("transposed") all the
way through; the host passes x pre-transposed [D, T, B_local] and
re-transposes the outputs, so the device never transposes the big tensors.
All transcendentals are sigmoid-only: tanh(z) = 2*sigmoid(2z) - 1, with the
factor 2 folded into the g-gate weights and the affine fix fused into the
DVE affine_mul_reduce op.

PSUM layout: two ping-pong tiles [128, 2048] (4 banks each); bank c holds
gate-chunk c for two consecutive steps x two batch subtiles, so the bias
and W_ih matmuls run at N=512 with one weight load per two steps.  Only
the W_hh matmuls (N=128) are per-step, as the recurrence requires.

Sharding: data-parallel over batch, 8 cores x 256 rows, weights replicated.
"""

import numpy as np

T = 31          # time steps (T_ref - 1)
D = 128         # input feature dim
H = 128         # hidden dim
G = 4 * H       # gate rows
NCORES = 8
B = 2048
BL = B // NCORES  # 256 batch rows per core
BS = 128          # batch sub-tile (2 per core)
NS = BL // BS
F32R = True       # use the fast fp32r PE path for matmuls (validated on HW)

_CACHE = {}


def _build_program(loop_n=0):
    from contextlib import ExitStack

    import concourse.bacc as bacc
    import concourse.mybir as mybir
    import concourse.tile as tile

    dt = mybir.dt.float32
    AF = mybir.ActivationFunctionType

    nc = bacc.Bacc("TRN2", target_bir_lowering=False, debug=False)

    xt_d = nc.dram_tensor("xt", [D, T, BL], dt, kind="ExternalInput").ap()
    wxb_d = nc.dram_tensor("wxb", [D, T], dt, kind="ExternalInput").ap()
    wih_d = nc.dram_tensor("wih", [D, G], dt, kind="ExternalInput").ap()
    whh_d = nc.dram_tensor("whh", [H, G], dt, kind="ExternalInput").ap()
    bias_d = nc.dram_tensor("bias", [2, G], dt, kind="ExternalInput").ap()
    ident_d = nc.dram_tensor("ident", [D, D], dt, kind="ExternalInput").ap()
    ones_d = nc.dram_tensor("ones", [2, 2 * BL], dt, kind="ExternalInput").ap()

    wt_d = nc.dram_tensor("wt_out", [T, D, BL], dt, kind="ExternalOutput").ap()
    enc_d = nc.dram_tensor("enc_out", [T, H, BL], dt, kind="ExternalOutput").ap()

    with ExitStack() as ctx:
        tc = ctx.enter_context(tile.TileContext(nc))

        def body():
            _emit(nc, tc, ctx, mybir, dt, AF,
                  xt_d, wxb_d, wih_d, whh_d, bias_d, ident_d, ones_d,
                  wt_d, enc_d)

        if loop_n:
            with tc.For_i(0, loop_n, 1):
                body()
        else:
            body()

    nc.compile()
    return nc


def _emit(nc, tc, ctx, mybir, dt, AF,
          xt_d, wxb_d, wih_d, whh_d, bias_d, ident_d, ones_d, wt_d, enc_d):
    from contextlib import ExitStack
    import concourse.bass as bass

    def rr(ap):
        return ap.bitcast(mybir.dt.float32r) if F32R else ap

    big = ctx.enter_context(tc.tile_pool(name="big", bufs=1))

    # ---- persistent SBUF tensors ----
    xt_s = big.tile([D, T * BL], dt, tag="xt")
    wid_s = big.tile([D, T * D], dt, tag="wid")
    wxb_s = big.tile([D, T], dt, tag="wxb")
    wxt_s = big.tile([D, T * BL], dt, tag="wxt")
    wxr_s = big.tile([D, T * BL], dt, tag="wxr")
    wih_s = big.tile([D, G], dt, tag="wih")
    whh_s = big.tile([H, G], dt, tag="whh")
    bias_s = big.tile([2, G], dt, tag="bias")
    ident_s = big.tile([D, D], dt, tag="ident")
    ones_s = big.tile([2, 2 * BL], dt, tag="ones")
    zro_s = big.tile([H, BS], dt, tag="zro")

    nc.sync.dma_start(out=ident_s[:], in_=ident_d[:])
    nc.sync.dma_start(out=wxb_s[:], in_=wxb_d[:])
    for t in range(T):
        nc.vector.tensor_scalar_mul(
            wid_s[:, t * D:(t + 1) * D], ident_s[:], wxb_s[:, t:t + 1])
    nc.vector.memset(zro_s[:], 0.0)

    # x input chunks next (x_score consumes them as they land); the LSTM
    # weights are not needed until the recurrence starts, so they go last.
    for t0 in range(0, T, 8):
        t1 = min(t0 + 8, T)
        nc.sync.dma_start(
            out=xt_s[:, t0 * BL:t1 * BL], in_=xt_d[:, t0:t1, :])
    nc.sync.dma_start(out=rr(wih_s[:]), in_=rr(wih_d[:]))
    nc.sync.dma_start(out=whh_s[:], in_=whh_d[:])
    nc.sync.dma_start(out=rr(bias_s[:]), in_=rr(bias_d[:]))
    nc.sync.dma_start(out=rr(ones_s[:]), in_=rr(ones_d[:]))

    with ExitStack() as fctx:
        fr = fctx.enter_context(tc.tile_pool(name="front", bufs=1))
        frs = fctx.enter_context(tc.tile_pool(name="fsmall", bufs=2))
        psf = fctx.enter_context(tc.tile_pool(name="psf", bufs=1, space="PSUM"))
        pst = fctx.enter_context(tc.tile_pool(name="pstr", bufs=2, space="PSUM"))

        # ---- PE warmup: transpose spins on ident until real work lands ----
        pwm = pst.tile([D, D], dt, tag="warm")
        for w in range(52):
            nc.tensor.transpose(pwm[:], ident_s[:], ident_s[:])

        # ---- x_score in natural [b, d]: ps_xs[j] += (xT_t chunk).T @ wid_t
        # (lhsT = x chunk so the product transposes x back; accumulate over t)
        ps_xs = [psf.tile([BS, D], dt, tag=f"xs{j}", name=f"ps_xs{j}")
                 for j in range(NS)]
        for t in range(T):
            for j in range(NS):
                nc.tensor.matmul(
                    ps_xs[j][:],
                    lhsT=xt_s[:, t * BL + j * BS: t * BL + (j + 1) * BS],
                    rhs=wid_s[:, t * D:(t + 1) * D],
                    start=(t == 0),
                    stop=(t == T - 1),
                )

        # ---- softmax straight off PSUM; transpose attn -> attnT ----
        attnT = big.tile([D, BL], dt, tag="attnT")
        for j in range(NS):
            nmx = frs.tile([BS, 1], dt, tag="nmx")
            nc.vector.tensor_reduce(
                nmx[:], ps_xs[j][:], axis=mybir.AxisListType.X,
                op=mybir.AluOpType.max, negate=True,
            )
            ex = frs.tile([BS, D], dt, tag="ex")
            sums = frs.tile([BS, 1], dt, tag="sums")
            nc.scalar.activation(ex[:], ps_xs[j][:], AF.Exp,
                                 bias=nmx[:], accum_out=sums[:])
            rc = frs.tile([BS, 1], dt, tag="rc")
            nc.vector.reciprocal(rc[:], sums[:])
            at = frs.tile([BS, D], dt, tag="at")
            nc.vector.tensor_scalar_mul(at[:], ex[:], rc[:])

            ptr2 = pst.tile([D, BS], dt, tag="ptr")
            nc.tensor.transpose(ptr2[:], at[:], ident_s[:])
            nc.vector.tensor_copy(attnT[:, j * BS:(j + 1) * BS], ptr2[:])

        # pre-trigger the Sigmoid table-set load while the front finishes
        sdum = frs.tile([BS, 1], dt, tag="sdum")
        nc.scalar.activation(sdum[:], nmx[:], AF.Sigmoid)

    # ---- LSTM recurrence ----
    # PSUM ping-pong tiles [128, 2048]: bank c = gate chunk c (pytorch order
    # i,f,g,o; g pre-scaled 2x), holding [s0_t | s1_t | s0_t+1 | s1_t+1].
    psg = ctx.enter_context(tc.tile_pool(name="psg", bufs=2, space="PSUM"))
    sgp = ctx.enter_context(tc.tile_pool(name="sg", bufs=6))
    sm = ctx.enter_context(tc.tile_pool(name="small", bufs=6))
    hst = ctx.enter_context(tc.tile_pool(name="hstage", bufs=3))
    jk = ctx.enter_context(tc.tile_pool(name="junk", bufs=4))

    c_prev = [zro_s, zro_s]
    h_prev = [zro_s, zro_s]
    hstage = None

    for tg in range(0, T, 2):  # 2-step groups
        gw = min(2, T - tg)                  # steps in this group
        nw = gw * BL                         # bias/W_ih matmul width
        # weighted input for this group: wxT_t = attnT * xT_t (exact fp32
        # for the wt output; fp32r rounded copy for the matmuls)
        for t in range(tg, tg + gw):
            nc.vector.tensor_mul(
                wxt_s[:, t * BL:(t + 1) * BL],
                xt_s[:, t * BL:(t + 1) * BL],
                attnT[:],
            )
            nc.vector.tensor_copy(
                rr(wxr_s[:, t * BL:(t + 1) * BL]),
                wxt_s[:, t * BL:(t + 1) * BL],
            )
        if tg % 8 == 6 or tg == 30:  # flush wt_out every 8 steps
            t0 = (tg // 8) * 8
            t1 = min(t0 + 8, T)
            nc.sync.dma_start(
                out=wt_d[t0:t1].rearrange("t d b -> d t b"),
                in_=wxt_s[:, t0 * BL:t1 * BL].rearrange(
                    "d (t b) -> d t b", b=BL),
            )
        ps = psg.tile([128, 4 * 512], dt, tag="gates")
        # bias + W_ih for both steps of the group, all 4 chunks, N=512
        for c in range(4):
            gseg = slice(c * H, (c + 1) * H)
            nc.tensor.matmul(
                ps[:, c * 512:c * 512 + nw], lhsT=rr(bias_s[0:2, gseg]),
                rhs=rr(ones_s[0:2, 0:nw]), start=True, stop=False,
                skip_group_check=True,
            )
            nc.tensor.matmul(
                ps[:, c * 512:c * 512 + nw], lhsT=rr(wih_s[:, gseg]),
                rhs=rr(wxr_s[:, tg * BL:tg * BL + nw]), start=False, stop=False,
                skip_group_check=True,
            )
        for dtw in range(gw):
            t = tg + dtw
            if t % 4 == 0:
                hstage = hst.tile([H, 4 * BL], dt, tag="hst")
            # phase A: both subtiles' W_hh matmuls + sigmoids, so the ACT
            # queue never has a pointwise-gated op ahead of a ready sigmoid
            sgs = []
            for s in range(NS):
                slot = dtw * 2 + s           # 128-col slot within each bank
                for c in range(4):
                    nc.tensor.matmul(
                        ps[:, c * 512 + slot * BS: c * 512 + (slot + 1) * BS],
                        lhsT=whh_s[:, c * H:(c + 1) * H],
                        rhs=h_prev[s][:],
                        start=False, stop=(slot == 2 * gw - 1),
                        skip_group_check=True,
                    )
                sg = sgp.tile([128, 4 * BS], dt, tag="sg", name=f"sg_{t}_{s}")
                ps_slot = ps[:].rearrange("p (c x) -> p c x", c=4)[
                    :, :, slot * BS:(slot + 1) * BS]
                nc.scalar.activation(sg[:], ps_slot, AF.Sigmoid)
                sgs.append(sg)
            # phase B: pointwise per subtile
            for s in range(NS):
                sg = sgs[s]
                si = sg[:, 0 * BS:1 * BS]
                sf = sg[:, 1 * BS:2 * BS]
                s2g = sg[:, 2 * BS:3 * BS]
                so = sg[:, 3 * BS:4 * BS]

                t1 = sm.tile([H, BS], dt, tag="t1", name=f"t1_{t}_{s}")
                nc.gpsimd.tensor_mul(t1[:], sf, c_prev[s][:])
                t2 = sm.tile([H, BS], dt, tag="t2", name=f"t2_{t}_{s}")
                j1 = jk.tile([H, 1], dt, tag="j1", name=f"j1_{t}_{s}")
                # t2 = tanh(g) * sigmoid(i) = (2*s2g - 1) * si
                nc.vector.affine_mul_reduce(
                    out=t2[:], accum_out=j1[:], in0=s2g, in1=si,
                    scale=2.0, bias=-1.0,
                )
                c_new = sm.tile([H, BS], dt, tag="c", name=f"c_{t}_{s}")
                nc.vector.tensor_add(c_new[:], t1[:], t2[:])
                s2c = sm.tile([H, BS], dt, tag="s2c", name=f"s2c_{t}_{s}")
                nc.scalar.activation(s2c[:], c_new[:], AF.Sigmoid, scale=2.0)
                h_new = hstage[:, (t % 4) * BL + s * BS:
                               (t % 4) * BL + (s + 1) * BS]
                j2 = jk.tile([H, 1], dt, tag="j2", name=f"j2_{t}_{s}")
                # h = tanh(c) * sigmoid(o) = (2*s2c - 1) * so
                nc.vector.affine_mul_reduce(
                    out=h_new, accum_out=j2[:], in0=s2c[:], in1=so,
                    scale=2.0, bias=-1.0,
                )
                c_prev[s] = c_new
                h_prev[s] = _Slice(h_new)
            if t % 4 == 3 or t == T - 1:
                t0 = (t // 4) * 4
                n = t - t0 + 1
                nc.sync.dma_start(
                    out=enc_d[t0:t0 + n].rearrange("t h b -> h t b"),
                    in_=hstage[:].rearrange("h (t b) -> h t b", t=4)[:, :n, :],
                )


class _Slice:
    """Tiny adapter so h_prev[s][:] works for both tiles and AP slices."""

    def __init__(self, ap):
        self._ap = ap

    def __getitem__(self, key):
        return self._ap


def _get_program():
    if "nc" not in _CACHE:
        _CACHE["nc"] = _build_program()
    return _CACHE["nc"]


def _trunc_fp32r(a):
    u = np.ascontiguousarray(a, np.float32).view(np.uint32)
    u = (u + 0x800) & np.uint32(0xFFFFF000)
    return u.view(np.float32)


def _host_inputs(input_data, W_ih, W_hh, b_ih, b_hh, attn_w, attn_b):
    """Build the per-core input maps (host-side prep is weights-only +
    layout transforms)."""
    x = np.ascontiguousarray(input_data, dtype=np.float32)
    W_ih = np.asarray(W_ih, dtype=np.float32)
    W_hh = np.asarray(W_hh, dtype=np.float32)
    b = (np.asarray(b_ih, dtype=np.float32)
         + np.asarray(b_hh, dtype=np.float32))
    w_x = np.asarray(attn_w, dtype=np.float32)[2 * H:]  # only the x-series part

    # scale the g-gate block (pytorch order i,f,g,o -> rows 2H:3H) by 2
    # so tanh(g) = 2*sigmoid(2g) - 1 works with a single sigmoid pass.
    scale = np.ones((G, 1), np.float32)
    scale[2 * H:3 * H] = 2.0
    wih_t = _trunc_fp32r(np.ascontiguousarray((W_ih * scale).T))  # [D, 4H]
    whh_t = np.ascontiguousarray((W_hh * scale).T)          # [H, 4H]
    bm = (b[None, :] * scale.T).astype(np.float32)
    b_hi = _trunc_fp32r(bm)
    b_lo = _trunc_fp32r(bm - b_hi)
    bias_m = np.ascontiguousarray(np.concatenate([b_hi, b_lo], 0))  # [2, 4H]

    wxb = np.ascontiguousarray(np.tile(w_x[None, :], (D, 1)))  # [D, T]
    ident = np.eye(D, dtype=np.float32)
    ones = np.ones((2, 2 * BL), np.float32)

    in_maps = []
    for i in range(NCORES):
        xs = x[i * BL:(i + 1) * BL]                  # [BL, T, D]
        xt = np.ascontiguousarray(xs.transpose(2, 1, 0))  # [D, T, BL]
        in_maps.append({
            "xt": xt,
            "wxb": wxb,
            "wih": wih_t,
            "whh": whh_t,
            "bias": bias_m,
            "ident": ident,
            "ones": ones,
        })
    return in_maps


def _gather(results):
    weighted = np.empty((B, T, D), np.float32)
    encoded = np.empty((B, T, H), np.float32)
    for i, r in enumerate(results):
        # wt_out/enc_out are [T, D|H, BL] -> [BL, T, D|H]
        weighted[i * BL:(i + 1) * BL] = r["wt_out"].transpose(2, 0, 1)
        encoded[i * BL:(i + 1) * BL] = r["enc_out"].transpose(2, 0, 1)
    return weighted, encoded


def kernel(input_data, W_ih, W_hh, b_ih, b_hh, attn_w, attn_b):
    from concourse.bass_utils import run_bass_kernel_spmd

    nc = _get_program()
    in_maps = _host_inputs(input_data, W_ih, W_hh, b_ih, b_hh, attn_w, attn_b)
    res = run_bass_kernel_spmd(nc, in_maps, list(range(NCORES)))
    return _gather(res.results)



# revision 6
# speedup vs baseline: 1.2863x; 1.2863x over previous
"""DA-RNN style encoder (LSTM + input attention) on 8 Trainium2 cores.

Problem: nn_Encoder_63024350101963
  B=2048, T-1=31 steps, D=128 input feats, H=128 hidden.

Key algebraic fact exploited: in the reference,
    score = (h @ w_h + c @ w_c + b)[:, None] + x_score
the recurrent term is constant along the softmax axis, and softmax is
shift-invariant, so
    attn = softmax(x_score)      (time-constant, recurrence-independent)
Therefore weighted = attn[:,None,:] * x  is a pure elementwise op and only
the LSTM cell recurrence is serial.

v2 design notes (vs the fp32 baseline):
  * fp16 everywhere except the PSUM accumulators and the softmax
    normalization: matmuls run at 1 cycle/row (vs 4 for fp32), DVE
    elementwise gets the 2x packed mode, and the x/wt/enc DMA traffic
    halves.  Accuracy budget is rel-err < 2e-2; fp16 lands ~1e-3.
  * tanh-only transcendentals: sigmoid(z) = 0.5*tanh(z/2)+0.5, applied with
    the ACT free scale (0.5) and the affine fused into affine_mul_reduce.
    tanh and exp share one ACT table set ("exp_and_others"), so the kernel
    performs ZERO activation-table switches (the fp32 baseline paid two
    ~1.3-2.7us loads per iteration for Exp->Sigmoid).
  * W_hh matmuls take both batch subtiles in one rhs [H, 256] (N=256,
    1 cycle/row) - 4 matmuls/step instead of 8.
  * bias is a single fp16 row via a K=1 matmul (no hi/lo split needed at
    this accuracy), W_ih keeps N=512 over 2-step groups.

Device layout: feature-on-partitions, batch-on-free ("transposed") all the
way through; the host passes x pre-transposed [D, T, B_local] (fp16) and
re-transposes/upcasts the outputs, so the device never transposes the big
tensors.

PSUM layout: two ping-pong tiles [128, 2048] (4 banks each); bank c holds
gate-chunk c (pytorch order i,f,g,o; g pre-scaled 2x) for two consecutive
steps x two batch subtiles, so the bias and W_ih matmuls run at N=512 with
one weight load per two steps.  Only the W_hh matmuls (N=256) are per-step,
as the recurrence requires.

Sharding: data-parallel over batch, 8 cores x 256 rows, weights replicated.
"""

import numpy as np

T = 31          # time steps (T_ref - 1)
D = 128         # input feature dim
H = 128         # hidden dim
G = 4 * H       # gate rows
NCORES = 8
B = 2048
BL = B // NCORES  # 256 batch rows per core
BS = 128          # batch sub-tile (2 per core)
NS = BL // BS
WARMUP = 40       # PE warm-up transposes (HAM ramp), overlapped with DMA

_CACHE = {}


def _build_program(loop_n=0):
    from contextlib import ExitStack

    import concourse.bacc as bacc
    import concourse.mybir as mybir
    import concourse.tile as tile

    f16 = mybir.dt.float16
    f32 = mybir.dt.float32

    nc = bacc.Bacc("TRN2", target_bir_lowering=False, debug=False)

    xt_d = nc.dram_tensor("xt", [D, T, BL], f16, kind="ExternalInput").ap()
    wid_d = nc.dram_tensor("wid", [D, T * D], f16, kind="ExternalInput").ap()
    wih_d = nc.dram_tensor("wih", [D, G], f16, kind="ExternalInput").ap()
    whh_d = nc.dram_tensor("whh", [H, G], f16, kind="ExternalInput").ap()
    bias_d = nc.dram_tensor("bias", [1, G], f16, kind="ExternalInput").ap()
    ident_d = nc.dram_tensor("ident", [D, D], f16, kind="ExternalInput").ap()
    ones_d = nc.dram_tensor("ones", [1, 2 * BL], f16, kind="ExternalInput").ap()

    wt_d = nc.dram_tensor("wt_out", [T, D, BL], f16, kind="ExternalOutput").ap()
    enc_d = nc.dram_tensor("enc_out", [T, H, BL], f16, kind="ExternalOutput").ap()

    with ExitStack() as ctx:
        tc = ctx.enter_context(tile.TileContext(nc))

        def body():
            _emit(nc, tc, ctx, mybir, f16, f32,
                  xt_d, wid_d, wih_d, whh_d, bias_d, ident_d, ones_d,
                  wt_d, enc_d)

        if loop_n:
            with tc.For_i(0, loop_n, 1):
                body()
        else:
            body()

    nc.compile()
    return nc


def _emit(nc, tc, ctx, mybir, f16, f32,
          xt_d, wid_d, wih_d, whh_d, bias_d, ident_d, ones_d, wt_d, enc_d):
    from contextlib import ExitStack

    AF = mybir.ActivationFunctionType

    big = ctx.enter_context(tc.tile_pool(name="big", bufs=1))

    # ---- persistent SBUF tensors ----
    xt_s = big.tile([D, T * BL], f16, tag="xt")
    wid_s = big.tile([D, T * D], f16, tag="wid")
    wxt_s = big.tile([D, T * BL], f16, tag="wxt")
    wih_s = big.tile([D, G], f16, tag="wih")
    whh_s = big.tile([H, G], f16, tag="whh")
    bias_s = big.tile([1, G], f16, tag="bias")
    ident_s = big.tile([D, D], f16, tag="ident")
    ones_s = big.tile([1, 2 * BL], f16, tag="ones")
    zro_s = big.tile([H, 2 * BS], f16, tag="zro")

    nc.sync.dma_start(out=ident_s[:], in_=ident_d[:])
    # wid = w_x[t] * I, built on host (fp16, 1MB) - diag stack for x_score.
    nc.sync.dma_start(out=wid_s[:], in_=wid_d[:])
    nc.vector.memset(zro_s[:], 0.0)

    # x input chunks next (x_score consumes them as they land); the LSTM
    # weights are not needed until the recurrence starts, so they go last.
    for t0 in range(0, T, 8):
        t1 = min(t0 + 8, T)
        nc.sync.dma_start(
            out=xt_s[:, t0 * BL:t1 * BL], in_=xt_d[:, t0:t1, :])
    nc.sync.dma_start(out=wih_s[:], in_=wih_d[:])
    nc.sync.dma_start(out=whh_s[:], in_=whh_d[:])
    nc.sync.dma_start(out=bias_s[:], in_=bias_d[:])
    nc.sync.dma_start(out=ones_s[:], in_=ones_d[:])

    attnT = big.tile([D, BL], f16, tag="attnT")

    with ExitStack() as fctx:
        frs = fctx.enter_context(tc.tile_pool(name="fsmall", bufs=2))
        psf = fctx.enter_context(tc.tile_pool(name="psf", bufs=1, space="PSUM"))
        pst = fctx.enter_context(tc.tile_pool(name="pstr", bufs=2, space="PSUM"))

        # ---- PE warmup: transpose spins on ident until real work lands ----
        pwm = pst.tile([D, D], f16, tag="warm")
        for w in range(WARMUP):
            nc.tensor.transpose(pwm[:], ident_s[:], ident_s[:])

        # ---- x_score in natural [b, d]: ps_xs[j] += (xT_t chunk).T @ wid_t
        # (lhsT = x chunk so the product transposes x back; accumulate over t)
        ps_xs = [psf.tile([BS, D], f32, tag=f"xs{j}", name=f"ps_xs{j}")
                 for j in range(NS)]
        for t in range(T):
            for j in range(NS):
                nc.tensor.matmul(
                    ps_xs[j][:],
                    lhsT=xt_s[:, t * BL + j * BS: t * BL + (j + 1) * BS],
                    rhs=wid_s[:, t * D:(t + 1) * D],
                    start=(t == 0),
                    stop=(t == T - 1),
                )

        # ---- softmax straight off PSUM; transpose attn -> attnT ----
        for j in range(NS):
            nmx = frs.tile([BS, 1], f32, tag="nmx")
            nc.vector.tensor_reduce(
                nmx[:], ps_xs[j][:], axis=mybir.AxisListType.X,
                op=mybir.AluOpType.max, negate=True,
            )
            ex = frs.tile([BS, D], f32, tag="ex")
            sums = frs.tile([BS, 1], f32, tag="sums")
            nc.scalar.activation(ex[:], ps_xs[j][:], AF.Exp,
                                 bias=nmx[:], accum_out=sums[:])
            rc = frs.tile([BS, 1], f32, tag="rc")
            nc.vector.reciprocal(rc[:], sums[:])
            at = frs.tile([BS, D], f16, tag="at")
            nc.vector.tensor_scalar_mul(at[:], ex[:], rc[:])

            ptr2 = pst.tile([D, BS], f16, tag="ptr")
            nc.tensor.transpose(ptr2[:], at[:], ident_s[:])
            nc.vector.tensor_copy(attnT[:, j * BS:(j + 1) * BS], ptr2[:])

    # ---- LSTM recurrence ----
    # PSUM ping-pong tiles [128, 2048]: bank c = gate chunk c (pytorch order
    # i,f,g,o; g pre-scaled 2x), holding [step tg (256 cols) | step tg+1].
    # All transcendentals are tanh: sigmoid(z) = 0.5*tanh(z/2)+0.5 with the
    # 0.5/0.5 affine fused into affine_mul_reduce; tanh shares the ACT table
    # set with the front's exp, so no table reloads ever happen.
    psg = ctx.enter_context(tc.tile_pool(name="psg", bufs=2, space="PSUM"))
    sgp = ctx.enter_context(tc.tile_pool(name="sg", bufs=6))
    sm = ctx.enter_context(tc.tile_pool(name="small", bufs=8))
    hst = ctx.enter_context(tc.tile_pool(name="hstage", bufs=3))

    mul = mybir.AluOpType.mult
    add = mybir.AluOpType.add

    # State scaling: the device carries CC = 2c and HH = 2h (W_hh pre-halved
    # on host; enc_out re-halved on host).  With sg = tanh(z/2) this makes
    # every pointwise op a single standard scalar_tensor_tensor (2x DVE mode):
    #   2*sigmoid(f)*c = 0.5*(tf+1)*CC,  2*sigmoid(i)*tanh(g) = (ti+1)*tg
    #   CC' = 0.5*A + B;  tanh(c) = tanh(0.5*CC');  HH = (to+1)*tanh(c)
    c_prev = [_Slice(zro_s[:, 0:BS]), _Slice(zro_s[:, BS:2 * BS])]
    h_prev_full = zro_s  # [H, 256] both subtiles, contiguous
    hstage = None
    ngroups = (T + 1) // 2

    def emit_head(g):
        """Group head: weighted input (GPSIMD, off the DVE chain), wt_out
        flush, PSUM tile alloc, bias + W_ih matmuls at N=512."""
        tg = 2 * g
        gw = min(2, T - tg)
        nw = gw * BL
        for t in range(tg, tg + gw):
            nc.gpsimd.tensor_mul(
                wxt_s[:, t * BL:(t + 1) * BL],
                xt_s[:, t * BL:(t + 1) * BL],
                attnT[:],
            )
        if tg % 8 == 6 or tg == 30:  # flush wt_out every 8 steps
            t0 = (tg // 8) * 8
            t1 = min(t0 + 8, T)
            nc.sync.dma_start(
                out=wt_d[t0:t1].rearrange("t d b -> d t b"),
                in_=wxt_s[:, t0 * BL:t1 * BL].rearrange(
                    "d (t b) -> d t b", b=BL),
            )
        ps = psg.tile([128, 4 * 512], f32, tag="gates", name=f"ps_{g}")
        for c in range(4):
            gseg = slice(c * H, (c + 1) * H)
            nc.tensor.matmul(
                ps[:, c * 512:c * 512 + nw], lhsT=bias_s[0:1, gseg],
                rhs=ones_s[0:1, 0:nw], start=True, stop=False,
                skip_group_check=True,
            )
            nc.tensor.matmul(
                ps[:, c * 512:c * 512 + nw], lhsT=wih_s[:, gseg],
                rhs=wxt_s[:, tg * BL:tg * BL + nw], start=False, stop=False,
                skip_group_check=True,
            )
        return ps

    ps_next = emit_head(0)
    for g in range(ngroups):
        tg = 2 * g
        gw = min(2, T - tg)
        ps = ps_next
        for dtw in range(gw):
            t = tg + dtw
            if t % 4 == 0:
                hstage = hst.tile([H, 4 * BL], f16, tag="hst")
            # W_hh for both subtiles in one rhs [H, 256] per chunk (N=256)
            for c in range(4):
                nc.tensor.matmul(
                    ps[:, c * 512 + dtw * BL: c * 512 + (dtw + 1) * BL],
                    lhsT=whh_s[:, c * H:(c + 1) * H],
                    rhs=h_prev_full[:],
                    start=False, stop=(dtw == gw - 1),
                    skip_group_check=True,
                )
            # Emit the NEXT group's head right after this group's first
            # W_hh burst: PE is otherwise idle during the ACT/DVE chain, and
            # this keeps those 8 matmuls OFF the critical h -> W_hh path.
            if dtw == 0 and g + 1 < ngroups:
                ps_next = emit_head(g + 1)
            # per-subtile tanh of the gates, then pointwise (phase-split so
            # s0's ACT/DVE work overlaps s1's)
            sgs = []
            for s in range(NS):
                slot = dtw * 2 + s           # 128-col slot within each bank
                sg = sgp.tile([128, 4 * BS], f16, tag="sg", name=f"sg_{t}_{s}")
                ps_slot = ps[:].rearrange("p (c x) -> p c x", c=4)[
                    :, :, slot * BS:(slot + 1) * BS]
                # sg = tanh(z/2): sigmoid(z) = 0.5*sg+0.5; g-rows pre-scaled
                # 2x on host so chunk 2 yields tanh(g) directly.
                nc.scalar.activation(sg[:], ps_slot, AF.Tanh, scale=0.5)
                sgs.append(sg)
            for s in range(NS):
                sg = sgs[s]
                ti_ = sg[:, 0 * BS:1 * BS]
                tf_ = sg[:, 1 * BS:2 * BS]
                tg_ = sg[:, 2 * BS:3 * BS]
                to_ = sg[:, 3 * BS:4 * BS]

                bv = sm.tile([H, BS], f16, tag="bv", name=f"bv_{t}_{s}")
                # B = (ti+1)*tg = 2*sigmoid(i)*tanh(g)
                nc.vector.scalar_tensor_tensor(
                    out=bv[:], in0=ti_, scalar=1.0, in1=tg_, op0=add, op1=mul)
                av = sm.tile([H, BS], f16, tag="av", name=f"av_{t}_{s}")
                # A = (tf+1)*CC = 4*sigmoid(f)*c
                nc.vector.scalar_tensor_tensor(
                    out=av[:], in0=tf_, scalar=1.0, in1=c_prev[s][:],
                    op0=add, op1=mul)
                c_new = sm.tile([H, BS], f16, tag="c", name=f"c_{t}_{s}")
                # CC' = 0.5*A + B
                nc.vector.scalar_tensor_tensor(
                    out=c_new[:], in0=av[:], scalar=0.5, in1=bv[:],
                    op0=mul, op1=add)
                tc_ = sm.tile([H, BS], f16, tag="tc", name=f"tc_{t}_{s}")
                nc.scalar.activation(tc_[:], c_new[:], AF.Tanh, scale=0.5)
                h_new = hstage[:, (t % 4) * BL + s * BS:
                               (t % 4) * BL + (s + 1) * BS]
                # HH = (to+1)*tanh(c) = 2*sigmoid(o)*tanh(c)
                nc.vector.scalar_tensor_tensor(
                    out=h_new, in0=to_, scalar=1.0, in1=tc_[:],
                    op0=add, op1=mul)
                c_prev[s] = c_new
            h_prev_full = _Slice(hstage[:, (t % 4) * BL:(t % 4 + 1) * BL])
            if t % 4 == 3 or t == T - 1:
                t0 = (t // 4) * 4
                n = t - t0 + 1
                nc.sync.dma_start(
                    out=enc_d[t0:t0 + n].rearrange("t h b -> h t b"),
                    in_=hstage[:].rearrange("h (t b) -> h t b", t=4)[:, :n, :],
                )


class _Slice:
    """Tiny adapter so h_prev_full[:] works for both tiles and AP slices."""

    def __init__(self, ap):
        self._ap = ap

    def __getitem__(self, key):
        return self._ap


def _get_program():
    if "nc" not in _CACHE:
        _CACHE["nc"] = _build_program()
    return _CACHE["nc"]


def _host_inputs(input_data, W_ih, W_hh, b_ih, b_hh, attn_w, attn_b):
    """Build the per-core input maps (host-side prep is weights-only +
    layout/dtype transforms)."""
    x = np.asarray(input_data, dtype=np.float32)
    W_ih = np.asarray(W_ih, dtype=np.float32)
    W_hh = np.asarray(W_hh, dtype=np.float32)
    b = (np.asarray(b_ih, dtype=np.float32)
         + np.asarray(b_hh, dtype=np.float32))
    w_x = np.asarray(attn_w, dtype=np.float32)[2 * H:]  # only the x-series part

    # scale the g-gate block (pytorch order i,f,g,o -> rows 2H:3H) by 2 so
    # tanh(z_g/2) = tanh(g) works with the single scale=0.5 tanh pass.
    scale = np.ones((G, 1), np.float32)
    scale[2 * H:3 * H] = 2.0
    wih_t = np.ascontiguousarray((W_ih * scale).T).astype(np.float16)  # [D, 4H]
    # device h-state is HH = 2h, so fold the 1/2 into W_hh
    whh_t = np.ascontiguousarray(
        (W_hh * scale).T * 0.5).astype(np.float16)                     # [H, 4H]
    bias_m = (b[None, :] * scale.T).astype(np.float16)                 # [1, 4H]

    # wid = stacked diag(w_x[t]) blocks [D, T*D], fp16
    wid = np.zeros((D, T * D), np.float16)
    idx = np.arange(D)
    for t in range(T):
        wid[idx, t * D + idx] = w_x[t].astype(np.float16)
    ident = np.eye(D, dtype=np.float16)
    ones = np.ones((1, 2 * BL), np.float16)

    in_maps = []
    for i in range(NCORES):
        xs = x[i * BL:(i + 1) * BL]                  # [BL, T, D]
        xt = np.ascontiguousarray(
            xs.transpose(2, 1, 0)).astype(np.float16)  # [D, T, BL]
        in_maps.append({
            "xt": xt,
            "wid": wid,
            "wih": wih_t,
            "whh": whh_t,
            "bias": bias_m,
            "ident": ident,
            "ones": ones,
        })
    return in_maps


def _gather(results):
    weighted = np.empty((B, T, D), np.float32)
    encoded = np.empty((B, T, H), np.float32)
    for i, r in enumerate(results):
        # wt_out/enc_out are [T, D|H, BL] fp16 -> [BL, T, D|H] fp32
        weighted[i * BL:(i + 1) * BL] = r["wt_out"].transpose(2, 0, 1)
        # enc_out carries HH = 2h
        encoded[i * BL:(i + 1) * BL] = (
            r["enc_out"].transpose(2, 0, 1).astype(np.float32) * 0.5)
    return weighted, encoded


def kernel(input_data, W_ih, W_hh, b_ih, b_hh, attn_w, attn_b):
    from concourse.bass_utils import run_bass_kernel_spmd

    nc = _get_program()
    in_maps = _host_inputs(input_data, W_ih, W_hh, b_ih, b_hh, attn_w, attn_b)
    res = run_bass_kernel_spmd(nc, in_maps, list(range(NCORES)))
    return _gather(res.results)
